# revision 24
# baseline (speedup 1.0000x reference)
"""DCNv2 deformable PS-RoI pooling on 8 Trainium2 NeuronCores.

Strategy (RoI-data-parallel, 32 rois per core, slot-capacity template):
  * Host replicates the reference coordinate math exactly in float32 and folds
    bilinear weights, validity masking and the 1/count normalization into a
    per-roi sparse matrix over the roi's exact touched pixel set (not its
    bbox hull).
  * Rois are snake-dealt to cores by touched-pixel count so the r-th roi of
    every core has a near-identical size; slot r gets a shared pixel capacity
    cap_r = max over cores.  Pixels pack contiguously across slots (no
    per-roi 128-padding; only the final chunk pads), so the gather volume is
    ~2.9k pixels/core instead of 5.5k.
  * Device (SPMD, one program, per-core data in DRAM inputs):
      - one indirect-DMA gather per slot-group: patch[:, t, :] holds 128
        pixels on partitions x 256 channels
      - per (chunk, slot) template block: 2 matmuls (channel halves) of
        patch_chunk(128px, 128c)^T @ A_block(128px, 49) accumulating into the
        group's PSUM bank pair with start/stop on the slot's first/last chunk
      - per group: PSUM -> SBUF copies (DVE for half 0, Act for half 1),
        then one DMA to HBM; host undoes the slot permutation.
"""
import numpy as np

f32 = np.float32
f64 = np.float64

B, C, H, W = 8, 256, 64, 64
N_ROIS, P, S = 256, 7, 4
PART = 7
NJ = P * P  # 49
SCALE = f32(1.0 / 16.0)
TRANS_STD = f32(0.1)
N_CORES = 8
RPC = N_ROIS // N_CORES  # rois (slots) per core
CH = 128  # chunk size (partition dim)
# slot-count per psum group (each group = 2 PSUM banks, <= 10 slots)
GROUP_SIZES = (4, 5, 5, 5, 5, 4, 4)
# gather i covers the chunks of groups GATHER_GROUPS[i]
GATHER_GROUPS = ((0,), (1, 2), (3, 4), (5, 6))
# pad a slot to the next chunk boundary when the gap is < ALIGN_PAD px
ALIGN_PAD = 48
IDX_SPLIT = True  # upload gather-1's indices in their own small DMA
N_AE = 2  # A uploaded eagerly for the first N_AE gathers; rest deferred
OUT_ENGINES = ("sync",)  # issuers for per-group out DMAs (in-order SEQ!)

_prog_cache = {}


# --------------------------------------------------------------------------
# host math: exact f32 replication of the reference coordinate computation
# --------------------------------------------------------------------------
def _roi_sampling_data(rois, offset):
    rois = np.asarray(rois, dtype=f32)
    offset = np.asarray(offset, dtype=f32)
    batch = rois[:, 0].astype(np.int32)

    roi_sw = np.round(rois[:, 1]) * SCALE - f32(0.5)
    roi_sh = np.round(rois[:, 2]) * SCALE - f32(0.5)
    roi_ew = (np.round(rois[:, 3]) + f32(1.0)) * SCALE - f32(0.5)
    roi_eh = (np.round(rois[:, 4]) + f32(1.0)) * SCALE - f32(0.5)
    roi_w = np.maximum(roi_ew - roi_sw, f32(0.1))
    roi_h = np.maximum(roi_eh - roi_sh, f32(0.1))
    bin_w = roi_w / f32(P)
    bin_h = roi_h / f32(P)
    sub_w = bin_w / f32(S)
    sub_h = bin_h / f32(S)

    ph = np.arange(P, dtype=np.int32)
    pw = np.arange(P, dtype=np.int32)
    part_h = np.clip(
        np.floor(ph.astype(f32) / f32(P) * f32(PART)).astype(np.int32), 0, PART - 1
    )
    part_w = np.clip(
        np.floor(pw.astype(f32) / f32(P) * f32(PART)).astype(np.int32), 0, PART - 1
    )

    tx = offset[:, 0][:, part_h[:, None], part_w[None, :]] * TRANS_STD  # (N,7,7)
    ty = offset[:, 1][:, part_h[:, None], part_w[None, :]] * TRANS_STD

    wstart = (
        pw.astype(f32)[None, None, :] * bin_w[:, None, None]
        + roi_sw[:, None, None]
        + tx * roi_w[:, None, None]
    )
    hstart = (
        ph.astype(f32)[None, :, None] * bin_h[:, None, None]
        + roi_sh[:, None, None]
        + ty * roi_h[:, None, None]
    )

    iw = np.arange(S, dtype=f32)
    ih = np.arange(S, dtype=f32)
    wpos = (
        wstart[:, :, :, None, None]
        + iw[None, None, None, None, :] * sub_w[:, None, None, None, None]
    )
    hpos = (
        hstart[:, :, :, None, None]
        + ih[None, None, None, :, None] * sub_h[:, None, None, None, None]
    )

    valid = (
        (wpos >= f32(-0.5)) & (wpos <= f32(W) - f32(0.5))
        & (hpos >= f32(-0.5)) & (hpos <= f32(H) - f32(0.5))
    )
    wc = np.clip(wpos, f32(0.0), f32(W - 1.0))
    hc = np.clip(hpos, f32(0.0), f32(H - 1.0))

    x0 = np.floor(wc).astype(np.int32)
    x1 = np.ceil(wc).astype(np.int32)
    y0 = np.floor(hc).astype(np.int32)
    y1 = np.ceil(hc).astype(np.int32)
    dx = (wc - np.floor(wc)).astype(f64)
    dy = (hc - np.floor(hc)).astype(f64)

    cnt = valid.sum(axis=(3, 4)).astype(f32)  # (N,7,7)
    coef = np.where(cnt > 0, 1.0 / np.maximum(cnt, f32(1.0)).astype(f64), 0.0)

    w00 = (1.0 - dx) * (1.0 - dy)
    w01 = dx * (1.0 - dy)
    w10 = (1.0 - dx) * dy
    w11 = dx * dy

    return dict(
        batch=batch, valid=valid, x0=x0, x1=x1, y0=y0, y1=y1,
        w00=w00, w01=w01, w10=w10, w11=w11, coef=coef,
    )


def _build_roi_sparse(rois, offset):
    """Per roi: (sorted unique pixel ids (npix,), A f64 (npix, 49))."""
    d = _roi_sampling_data(rois, offset)
    full = (P, P, S, S)
    j_grid = np.broadcast_to(
        np.arange(NJ, dtype=np.int64).reshape(P, P, 1, 1), full
    )

    out = []
    for n in range(N_ROIS):
        v = d["valid"][n]
        if not v.any():
            out.append((np.zeros(0, np.int32), np.zeros((0, NJ), f64)))
            continue
        jj = j_grid[v]
        b = int(d["batch"][n])
        ids = []
        ws = []
        for xk, yk, wk in (
            ("x0", "y0", "w00"), ("x1", "y0", "w01"),
            ("x0", "y1", "w10"), ("x1", "y1", "w11"),
        ):
            xs = np.broadcast_to(d[xk][n], full)[v].astype(np.int64)
            ys = np.broadcast_to(d[yk][n], full)[v].astype(np.int64)
            cf = np.broadcast_to(d["coef"][n][:, :, None, None], full)[v]
            ids.append(b * (H * W) + ys * W + xs)
            ws.append(np.broadcast_to(d[wk][n], full)[v] * cf)
        ids = np.concatenate(ids)
        ws = np.concatenate(ws)
        jjs = np.concatenate([jj] * 4)
        uniq, inv = np.unique(ids, return_inverse=True)
        A = np.zeros((len(uniq), NJ), f64)
        np.add.at(A, (inv, jjs), ws)
        out.append((uniq.astype(np.int32), A))
    return out


# --------------------------------------------------------------------------
# planning: snake deal -> slot capacities -> chunk/block template
# --------------------------------------------------------------------------
def _plan(sizes):
    """sizes: (256,) touched-pixel counts. Returns slots (core->slot->roi),
    caps, and the template (chunks, blocks, groups)."""
    order = np.argsort(-sizes, kind="stable")
    ranks = [[None] * RPC for _ in range(N_CORES)]  # ranks[k][rank] = roi
    for i, roi in enumerate(order):
        rnd, pos = divmod(i, N_CORES)
        core = pos if rnd % 2 == 0 else N_CORES - 1 - pos
        ranks[core][rnd] = int(roi)

    # rank (0 = largest) -> slot position: first group = small rois (fast
    # pipeline start), middle groups = descending sizes, last group = smallest
    n0 = GROUP_SIZES[0]
    nlast = GROUP_SIZES[-1]
    g0_ranks = list(range(RPC - nlast - n0, RPC - nlast))
    glast_ranks = list(range(RPC - nlast, RPC))
    mid_ranks = list(range(0, RPC - nlast - n0))
    slot_rank = g0_ranks + mid_ranks + glast_ranks
    assert sorted(slot_rank) == list(range(RPC))
    slots = [[ranks[k][slot_rank[r]] for r in range(RPC)] for k in range(N_CORES)]

    caps = []
    for r in range(RPC):
        cap = max(int(sizes[slots[k][r]]) for k in range(N_CORES))
        caps.append(cap)

    # alignment padding: if the next slot would start < ALIGN_PAD px before a
    # chunk boundary, round up (trades a few gathered pixels for one fewer
    # 49-col A block + matmul pair)
    pos = []
    off = 0
    for r in range(RPC):
        pos.append(off)
        off += caps[r]
        if ALIGN_PAD and r + 1 < RPC:
            tail = off % CH
            if tail and CH - tail < ALIGN_PAD:
                off += CH - tail  # next slot starts at a chunk boundary
    total = off
    T = -(-total // CH)

    # blocks: (t, slot) incidences in global (t, slot) order so the device
    # consumes chunks in gather-arrival order; aoff follows emission order
    gb = np.cumsum((0,) + GROUP_SIZES)
    raw = []
    for r in range(RPC):
        if caps[r] == 0:
            continue
        t_first = pos[r] // CH
        t_last = (pos[r] + caps[r] - 1) // CH
        for t in range(t_first, t_last + 1):
            raw.append((t, r, t == t_first, t == t_last))
    raw.sort(key=lambda b: (b[0], b[1]))
    blocks = []
    aoff = 0
    for (t, r, st, sp) in raw:
        blocks.append((t, r, aoff, st, sp))
        aoff += NJ
    acols = aoff
    group_of = np.searchsorted(gb[1:], np.arange(RPC), side="right")
    groups = [(int(gb[g]), int(gb[g + 1])) for g in range(len(GROUP_SIZES))]

    # gather ranges: each gather covers the chunks of its GATHER_GROUPS
    assert tuple(sorted(g for gs in GATHER_GROUPS for g in gs)) == tuple(
        range(len(GROUP_SIZES))
    )
    cuts = [0]
    for gi, gs in enumerate(GATHER_GROUPS):
        glast = max(gs)
        s1 = int(gb[glast + 1])
        end = pos[s1 - 1] + caps[s1 - 1]
        t_hi = T if gi == len(GATHER_GROUPS) - 1 else min(-(-end // CH), T)
        cuts.append(max(t_hi, cuts[-1]))
    if cuts[-1] != T:
        cuts[-1] = T
    granges = [(cuts[i], cuts[i + 1]) for i in range(len(cuts) - 1)
               if cuts[i + 1] > cuts[i]]
    assert granges[-1][1] == T

    return dict(slots=slots, caps=caps, pos=pos, T=T, acols=acols,
                blocks=blocks, group_of=group_of, groups=groups,
                granges=granges)


def _plan_key(plan):
    return (tuple(plan["caps"]), plan["T"], plan["acols"],
            tuple(plan["granges"]), tuple(plan["blocks"]),
            tuple(int(g) for g in plan["group_of"]))


# --------------------------------------------------------------------------
# device program
# --------------------------------------------------------------------------
def _build_program(plan):
    import concourse.bacc as bacc
    import concourse.mybir as mybir
    from concourse.tile import TileContext

    T = plan["T"]
    acols = plan["acols"]
    blocks = plan["blocks"]
    group_of = plan["group_of"]
    groups = plan["groups"]
    granges = plan["granges"]

    nc = bacc.Bacc("TRN2", num_devices=N_CORES)
    dt = mybir.dt
    fcl = nc.dram_tensor("fcl", [B * H * W, C], dt.float16, kind="ExternalInput")
    amat = nc.dram_tensor("amat", [128, acols], dt.float16, kind="ExternalInput")
    pidx = nc.dram_tensor("pidx", [128, T * 8], dt.int16, kind="ExternalInput")
    outd = nc.dram_tensor("out", [128, RPC, 2, NJ], dt.float16, kind="ExternalOutput")

    # last block index per group (emission order = blocks order)
    last_of = {}
    for i, (t, r, aoff, st, sp) in enumerate(blocks):
        last_of[int(group_of[r])] = i

    # split the A upload: an early piece covering the first N_AE gathers'
    # blocks, and a deferred rest issued behind group-0's out DMA so it stays
    # off the early DMA queue
    ng = len(granges)
    need = [0] * ng  # cols needed through gather i
    for (t, r, aoff, st, sp) in blocks:
        for i, (t0, t1) in enumerate(granges):
            if t < t1:
                need[i] = max(need[i], aoff + NJ)
                break
    for i in range(1, ng):
        need[i] = max(need[i], need[i - 1])
    a_early = min(max(need[min(N_AE - 1, ng - 1)], NJ), acols)

    icut = granges[0][1] * 8 if (IDX_SPLIT and len(granges) > 1) else T * 8

    with TileContext(nc) as tc:
        with (
            tc.tile_pool(name="main", bufs=1) as mp,
            tc.tile_pool(name="psum", bufs=1, space="PSUM") as pp,
        ):
            idx_t = mp.tile([128, T * 8], dt.int16, tag="idx")
            nc.sync.dma_start(out=idx_t[:, :icut], in_=pidx[:, :icut])
            if icut < T * 8:
                nc.sync.dma_start(out=idx_t[:, icut:], in_=pidx[:, icut:])
            a_t = mp.tile([128, acols], dt.float16, tag="amat")
            nc.sync.dma_start(out=a_t[:, :a_early], in_=amat[:, :a_early])
            patch = mp.tile([128, T, C], dt.float16, tag="patch")

            for (t0, t1) in granges:
                nc.gpsimd.dma_gather(
                    out_ap=patch[:, t0:t1, :],
                    in_ap=fcl[:],
                    idxs_ap=idx_t[:, t0 * 8:t1 * 8],
                    num_idxs=(t1 - t0) * 128,
                    num_idxs_reg=(t1 - t0) * 128,
                    elem_size=C,
                    single_packet=False,
                )

            pbs = {}
            for i, (t, r, aoff, st, sp) in enumerate(blocks):
                g = int(group_of[r])
                s0, s1 = groups[g]
                ns = s1 - s0
                if g not in pbs:
                    pbs[g] = [
                        pp.tile([128, ns, NJ], dt.float32, tag=f"pb{g % 4}_{h}",
                                name=f"pb{g}_{h}")
                        for h in range(2)
                    ]
                for h in range(2):
                    nc.tensor.matmul(
                        out=pbs[g][h][:, r - s0, :],
                        lhsT=patch[:, t, h * 128:(h + 1) * 128],
                        rhs=a_t[:, aoff:aoff + NJ],
                        start=st,
                        stop=sp,
                    )
                if i == last_of[g]:
                    ob = mp.tile([128, ns, 2, NJ], dt.float16, tag=f"ob{g}",
                                 name=f"ob{g}")
                    nc.vector.tensor_copy(out=ob[:, :, 0, :], in_=pbs[g][0][:])
                    nc.scalar.copy(out=ob[:, :, 1, :], in_=pbs[g][1][:])
                    eng = getattr(nc, OUT_ENGINES[g % len(OUT_ENGINES)])
                    eng.dma_start(out=outd[:, s0:s1, :, :], in_=ob[:])
                    if g == 0 and a_early < acols:
                        # deferred bulk A upload: SP's in-order SEQ holds this
                        # behind out-g0's sem wait, keeping it off the DMA
                        # queue while the early gathers stream
                        nc.sync.dma_start(out=a_t[:, a_early:],
                                          in_=amat[:, a_early:])
    nc.compile()
    return nc


# --------------------------------------------------------------------------
# entry point
# --------------------------------------------------------------------------
def kernel(input, rois, offset):
    from concourse.bass_utils import run_bass_kernel_spmd

    input = np.asarray(input, dtype=f32)
    mats = _build_roi_sparse(rois, offset)
    sizes = np.array([len(g) for g, _ in mats])
    plan = _plan(sizes)

    key = _plan_key(plan)
    if key not in _prog_cache:
        _prog_cache[key] = _build_program(plan)
    nc = _prog_cache[key]

    fcl = np.ascontiguousarray(
        input.transpose(0, 2, 3, 1).astype(np.float16)
    ).reshape(B * H * W, C)

    T, acols = plan["T"], plan["acols"]
    caps, pos, slots = plan["caps"], plan["pos"], plan["slots"]

    in_maps = []
    for k in range(N_CORES):
        logical = np.zeros(T * CH, np.int32)
        a_arr = np.zeros((128, acols), np.float16)
        for (t, r, aoff, st, sp) in plan["blocks"]:
            gidx, A = mats[slots[k][r]]
            npix = len(gidx)
            lo = max(t * CH, pos[r])
            hi = min((t + 1) * CH, pos[r] + npix)
            if hi <= lo:
                continue
            i0 = lo - pos[r]
            i1 = hi - pos[r]
            logical[lo:hi] = gidx[i0:i1]
            a_arr[lo - t * CH:hi - t * CH, aoff:aoff + NJ] = (
                A[i0:i1].astype(np.float16)
            )
        idx16 = np.tile(logical.astype(np.int16).reshape(-1, 16).T, (8, 1))
        in_maps.append({"fcl": fcl, "amat": a_arr, "pidx": idx16})

    res = run_bass_kernel_spmd(nc, in_maps, core_ids=list(range(N_CORES)))

    out_full = np.empty((N_ROIS, C, P, P), f32)
    for k in range(N_CORES):
        arr = res.results[k]["out"].astype(f32)  # (128, RPC, 2, 49)
        t = arr.transpose(1, 2, 0, 3).reshape(RPC, C, P, P)
        for r in range(RPC):
            roi = slots[k][r]
            if len(mats[roi][0]) == 0 and caps[r] > 0:
                out_full[roi] = 0.0
            elif caps[r] == 0:
                out_full[roi] = 0.0
            else:
                out_full[roi] = t[r]
    return out_full


# revision 27
# speedup vs baseline: 1.0712x; 1.0712x over previous
"""DCNv2 deformable PS-RoI pooling on 8 Trainium2 NeuronCores.

Strategy (RoI-data-parallel, 32 rois per core, slot-capacity template):
  * Host replicates the reference coordinate math exactly in float32 and folds
    bilinear weights, validity masking and the 1/count normalization into a
    per-roi sparse matrix over the roi's exact touched pixel set (not its
    bbox hull).
  * Rois are snake-dealt to cores by touched-pixel count so the r-th roi of
    every core has a near-identical size; slot r gets a shared pixel capacity
    cap_r = max over cores.  Pixels pack contiguously across slots (no
    per-roi 128-padding; only the final chunk pads), so the gather volume is
    ~2.9k pixels/core instead of 5.5k.
  * Device (SPMD, one program, per-core data in DRAM inputs):
      - one indirect-DMA gather per slot-group: patch[:, t, :] holds 128
        pixels on partitions x 256 channels
      - per (chunk, slot) template block: 2 matmuls (channel halves) of
        patch_chunk(128px, 128c)^T @ A_block(128px, 49) accumulating into the
        group's PSUM bank pair with start/stop on the slot's first/last chunk
      - per group: PSUM -> SBUF copies (DVE for half 0, Act for half 1),
        then one DMA to HBM; host undoes the slot permutation.
"""
import numpy as np

f32 = np.float32
f64 = np.float64

B, C, H, W = 8, 256, 64, 64
N_ROIS, P, S = 256, 7, 4
PART = 7
NJ = P * P  # 49
SCALE = f32(1.0 / 16.0)
TRANS_STD = f32(0.1)
N_CORES = 8
RPC = N_ROIS // N_CORES  # rois (slots) per core
CH = 128  # chunk size (partition dim)
# slot-count per psum group (each group = 2 PSUM banks, <= 10 slots)
GROUP_SIZES = (4, 10, 10, 8)
# gather i covers the chunks of groups GATHER_GROUPS[i]
GATHER_GROUPS = ((0,), (1,), (2,), (3,))
# pad a slot to the next chunk boundary when the gap is < ALIGN_PAD px
ALIGN_PAD = 0
IDX_SPLIT = False  # upload gather-1's indices in their own small DMA
N_AE = 1  # A uploaded eagerly for the first N_AE gathers; rest eager too
A_DEFER = False  # defer the second A piece behind out-g0
OUT_ENGINES = ("sync",)  # issuers for per-group out DMAs (in-order SEQ!)

_prog_cache = {}


# --------------------------------------------------------------------------
# host math: exact f32 replication of the reference coordinate computation
# --------------------------------------------------------------------------
def _roi_sampling_data(rois, offset):
    rois = np.asarray(rois, dtype=f32)
    offset = np.asarray(offset, dtype=f32)
    batch = rois[:, 0].astype(np.int32)

    roi_sw = np.round(rois[:, 1]) * SCALE - f32(0.5)
    roi_sh = np.round(rois[:, 2]) * SCALE - f32(0.5)
    roi_ew = (np.round(rois[:, 3]) + f32(1.0)) * SCALE - f32(0.5)
    roi_eh = (np.round(rois[:, 4]) + f32(1.0)) * SCALE - f32(0.5)
    roi_w = np.maximum(roi_ew - roi_sw, f32(0.1))
    roi_h = np.maximum(roi_eh - roi_sh, f32(0.1))
    bin_w = roi_w / f32(P)
    bin_h = roi_h / f32(P)
    sub_w = bin_w / f32(S)
    sub_h = bin_h / f32(S)

    ph = np.arange(P, dtype=np.int32)
    pw = np.arange(P, dtype=np.int32)
    part_h = np.clip(
        np.floor(ph.astype(f32) / f32(P) * f32(PART)).astype(np.int32), 0, PART - 1
    )
    part_w = np.clip(
        np.floor(pw.astype(f32) / f32(P) * f32(PART)).astype(np.int32), 0, PART - 1
    )

    tx = offset[:, 0][:, part_h[:, None], part_w[None, :]] * TRANS_STD  # (N,7,7)
    ty = offset[:, 1][:, part_h[:, None], part_w[None, :]] * TRANS_STD

    wstart = (
        pw.astype(f32)[None, None, :] * bin_w[:, None, None]
        + roi_sw[:, None, None]
        + tx * roi_w[:, None, None]
    )
    hstart = (
        ph.astype(f32)[None, :, None] * bin_h[:, None, None]
        + roi_sh[:, None, None]
        + ty * roi_h[:, None, None]
    )

    iw = np.arange(S, dtype=f32)
    ih = np.arange(S, dtype=f32)
    wpos = (
        wstart[:, :, :, None, None]
        + iw[None, None, None, None, :] * sub_w[:, None, None, None, None]
    )
    hpos = (
        hstart[:, :, :, None, None]
        + ih[None, None, None, :, None] * sub_h[:, None, None, None, None]
    )

    valid = (
        (wpos >= f32(-0.5)) & (wpos <= f32(W) - f32(0.5))
        & (hpos >= f32(-0.5)) & (hpos <= f32(H) - f32(0.5))
    )
    wc = np.clip(wpos, f32(0.0), f32(W - 1.0))
    hc = np.clip(hpos, f32(0.0), f32(H - 1.0))

    x0 = np.floor(wc).astype(np.int32)
    x1 = np.ceil(wc).astype(np.int32)
    y0 = np.floor(hc).astype(np.int32)
    y1 = np.ceil(hc).astype(np.int32)
    dx = (wc - np.floor(wc)).astype(f64)
    dy = (hc - np.floor(hc)).astype(f64)

    cnt = valid.sum(axis=(3, 4)).astype(f32)  # (N,7,7)
    coef = np.where(cnt > 0, 1.0 / np.maximum(cnt, f32(1.0)).astype(f64), 0.0)

    w00 = (1.0 - dx) * (1.0 - dy)
    w01 = dx * (1.0 - dy)
    w10 = (1.0 - dx) * dy
    w11 = dx * dy

    return dict(
        batch=batch, valid=valid, x0=x0, x1=x1, y0=y0, y1=y1,
        w00=w00, w01=w01, w10=w10, w11=w11, coef=coef,
    )


def _build_roi_sparse(rois, offset):
    """Per roi: (sorted unique pixel ids (npix,), A f64 (npix, 49))."""
    d = _roi_sampling_data(rois, offset)
    full = (P, P, S, S)
    j_grid = np.broadcast_to(
        np.arange(NJ, dtype=np.int64).reshape(P, P, 1, 1), full
    )

    out = []
    for n in range(N_ROIS):
        v = d["valid"][n]
        if not v.any():
            out.append((np.zeros(0, np.int32), np.zeros((0, NJ), f64)))
            continue
        jj = j_grid[v]
        b = int(d["batch"][n])
        ids = []
        ws = []
        for xk, yk, wk in (
            ("x0", "y0", "w00"), ("x1", "y0", "w01"),
            ("x0", "y1", "w10"), ("x1", "y1", "w11"),
        ):
            xs = np.broadcast_to(d[xk][n], full)[v].astype(np.int64)
            ys = np.broadcast_to(d[yk][n], full)[v].astype(np.int64)
            cf = np.broadcast_to(d["coef"][n][:, :, None, None], full)[v]
            ids.append(b * (H * W) + ys * W + xs)
            ws.append(np.broadcast_to(d[wk][n], full)[v] * cf)
        ids = np.concatenate(ids)
        ws = np.concatenate(ws)
        jjs = np.concatenate([jj] * 4)
        uniq, inv = np.unique(ids, return_inverse=True)
        A = np.zeros((len(uniq), NJ), f64)
        np.add.at(A, (inv, jjs), ws)
        out.append((uniq.astype(np.int32), A))
    return out


# --------------------------------------------------------------------------
# planning: snake deal -> slot capacities -> chunk/block template
# --------------------------------------------------------------------------
def _plan(sizes):
    """sizes: (256,) touched-pixel counts. Returns slots (core->slot->roi),
    caps, and the template (chunks, blocks, groups)."""
    order = np.argsort(-sizes, kind="stable")
    ranks = [[None] * RPC for _ in range(N_CORES)]  # ranks[k][rank] = roi
    for i, roi in enumerate(order):
        rnd, pos = divmod(i, N_CORES)
        core = pos if rnd % 2 == 0 else N_CORES - 1 - pos
        ranks[core][rnd] = int(roi)

    # rank (0 = largest) -> slot position: first group = small rois (fast
    # pipeline start), middle groups = descending sizes, last group = smallest
    n0 = GROUP_SIZES[0]
    nlast = GROUP_SIZES[-1]
    g0_ranks = list(range(RPC - nlast - n0, RPC - nlast))
    glast_ranks = list(range(RPC - nlast, RPC))
    mid_ranks = list(range(0, RPC - nlast - n0))
    slot_rank = g0_ranks + mid_ranks + glast_ranks
    assert sorted(slot_rank) == list(range(RPC))
    slots = [[ranks[k][slot_rank[r]] for r in range(RPC)] for k in range(N_CORES)]

    caps = []
    for r in range(RPC):
        cap = max(int(sizes[slots[k][r]]) for k in range(N_CORES))
        caps.append(cap)

    # alignment padding: if the next slot would start < ALIGN_PAD px before a
    # chunk boundary, round up (trades a few gathered pixels for one fewer
    # 49-col A block + matmul pair)
    pos = []
    off = 0
    for r in range(RPC):
        pos.append(off)
        off += caps[r]
        if ALIGN_PAD and r + 1 < RPC:
            tail = off % CH
            if tail and CH - tail < ALIGN_PAD:
                off += CH - tail  # next slot starts at a chunk boundary
    total = off
    T = -(-total // CH)

    # blocks: (t, slot) incidences in global (t, slot) order so the device
    # consumes chunks in gather-arrival order; aoff follows emission order
    gb = np.cumsum((0,) + GROUP_SIZES)
    raw = []
    for r in range(RPC):
        if caps[r] == 0:
            continue
        t_first = pos[r] // CH
        t_last = (pos[r] + caps[r] - 1) // CH
        for t in range(t_first, t_last + 1):
            raw.append((t, r, t == t_first, t == t_last))
    raw.sort(key=lambda b: (b[0], b[1]))
    blocks = []
    aoff = 0
    for (t, r, st, sp) in raw:
        blocks.append((t, r, aoff, st, sp))
        aoff += NJ
    acols = aoff
    group_of = np.searchsorted(gb[1:], np.arange(RPC), side="right")
    groups = [(int(gb[g]), int(gb[g + 1])) for g in range(len(GROUP_SIZES))]

    # gather ranges: each gather covers the chunks of its GATHER_GROUPS
    assert tuple(sorted(g for gs in GATHER_GROUPS for g in gs)) == tuple(
        range(len(GROUP_SIZES))
    )
    cuts = [0]
    for gi, gs in enumerate(GATHER_GROUPS):
        glast = max(gs)
        s1 = int(gb[glast + 1])
        end = pos[s1 - 1] + caps[s1 - 1]
        t_hi = T if gi == len(GATHER_GROUPS) - 1 else min(-(-end // CH), T)
        cuts.append(max(t_hi, cuts[-1]))
    if cuts[-1] != T:
        cuts[-1] = T
    granges = [(cuts[i], cuts[i + 1]) for i in range(len(cuts) - 1)
               if cuts[i + 1] > cuts[i]]
    assert granges[-1][1] == T

    return dict(slots=slots, caps=caps, pos=pos, T=T, acols=acols,
                blocks=blocks, group_of=group_of, groups=groups,
                granges=granges)


def _plan_key(plan):
    return (tuple(plan["caps"]), plan["T"], plan["acols"],
            tuple(plan["granges"]), tuple(plan["blocks"]),
            tuple(int(g) for g in plan["group_of"]))


# --------------------------------------------------------------------------
# device program
# --------------------------------------------------------------------------
def _build_program(plan):
    import concourse.bacc as bacc
    import concourse.mybir as mybir
    from concourse.tile import TileContext

    T = plan["T"]
    acols = plan["acols"]
    blocks = plan["blocks"]
    group_of = plan["group_of"]
    groups = plan["groups"]
    granges = plan["granges"]

    nc = bacc.Bacc("TRN2", num_devices=N_CORES)
    dt = mybir.dt
    fcl = nc.dram_tensor("fcl", [B * H * W, C], dt.float16, kind="ExternalInput")
    amat = nc.dram_tensor("amat", [128, acols], dt.float16, kind="ExternalInput")
    pidx = nc.dram_tensor("pidx", [128, T * 8], dt.int16, kind="ExternalInput")
    outd = nc.dram_tensor("out", [128, RPC, 2, NJ], dt.float16, kind="ExternalOutput")

    # last block index per group (emission order = blocks order)
    last_of = {}
    for i, (t, r, aoff, st, sp) in enumerate(blocks):
        last_of[int(group_of[r])] = i

    # split the A upload: an early piece covering the first N_AE gathers'
    # blocks, and a deferred rest issued behind group-0's out DMA so it stays
    # off the early DMA queue
    ng = len(granges)
    need = [0] * ng  # cols needed through gather i
    for (t, r, aoff, st, sp) in blocks:
        for i, (t0, t1) in enumerate(granges):
            if t < t1:
                need[i] = max(need[i], aoff + NJ)
                break
    for i in range(1, ng):
        need[i] = max(need[i], need[i - 1])
    a_early = min(max(need[min(N_AE - 1, ng - 1)], NJ), acols)

    icut = granges[0][1] * 8 if (IDX_SPLIT and len(granges) > 1) else T * 8

    with TileContext(nc) as tc:
        with (
            tc.tile_pool(name="main", bufs=1) as mp,
            tc.tile_pool(name="psum", bufs=1, space="PSUM") as pp,
        ):
            idx_t = mp.tile([128, T * 8], dt.int16, tag="idx")
            nc.sync.dma_start(out=idx_t[:, :icut], in_=pidx[:, :icut])
            if icut < T * 8:
                nc.sync.dma_start(out=idx_t[:, icut:], in_=pidx[:, icut:])
            a_t = mp.tile([128, acols], dt.float16, tag="amat")
            nc.sync.dma_start(out=a_t[:, :a_early], in_=amat[:, :a_early])
            if not A_DEFER and a_early < acols:
                nc.sync.dma_start(out=a_t[:, a_early:], in_=amat[:, a_early:])
            patch = mp.tile([128, T, C], dt.float16, tag="patch")

            for (t0, t1) in granges:
                nc.gpsimd.dma_gather(
                    out_ap=patch[:, t0:t1, :],
                    in_ap=fcl[:],
                    idxs_ap=idx_t[:, t0 * 8:t1 * 8],
                    num_idxs=(t1 - t0) * 128,
                    num_idxs_reg=(t1 - t0) * 128,
                    elem_size=C,
                    single_packet=False,
                )

            pbs = {}
            for i, (t, r, aoff, st, sp) in enumerate(blocks):
                g = int(group_of[r])
                s0, s1 = groups[g]
                ns = s1 - s0
                if g not in pbs:
                    pbs[g] = [
                        pp.tile([128, ns, NJ], dt.float32, tag=f"pb{g % 4}_{h}",
                                name=f"pb{g}_{h}")
                        for h in range(2)
                    ]
                for h in range(2):
                    nc.tensor.matmul(
                        out=pbs[g][h][:, r - s0, :],
                        lhsT=patch[:, t, h * 128:(h + 1) * 128],
                        rhs=a_t[:, aoff:aoff + NJ],
                        start=st,
                        stop=sp,
                    )
                if i == last_of[g]:
                    ob = mp.tile([128, ns, 2, NJ], dt.float16, tag=f"ob{g}",
                                 name=f"ob{g}")
                    nc.vector.tensor_copy(out=ob[:, :, 0, :], in_=pbs[g][0][:])
                    nc.scalar.copy(out=ob[:, :, 1, :], in_=pbs[g][1][:])
                    eng = getattr(nc, OUT_ENGINES[g % len(OUT_ENGINES)])
                    eng.dma_start(out=outd[:, s0:s1, :, :], in_=ob[:])
                    if A_DEFER and g == 0 and a_early < acols:
                        # deferred bulk A upload: SP's in-order SEQ holds this
                        # behind out-g0's sem wait, keeping it off the DMA
                        # queue while the early gathers stream
                        nc.sync.dma_start(out=a_t[:, a_early:],
                                          in_=amat[:, a_early:])
    nc.compile()
    return nc


# --------------------------------------------------------------------------
# entry point
# --------------------------------------------------------------------------
def kernel(input, rois, offset):
    from concourse.bass_utils import run_bass_kernel_spmd

    input = np.asarray(input, dtype=f32)
    mats = _build_roi_sparse(rois, offset)
    sizes = np.array([len(g) for g, _ in mats])
    plan = _plan(sizes)

    key = _plan_key(plan)
    if key not in _prog_cache:
        _prog_cache[key] = _build_program(plan)
    nc = _prog_cache[key]

    fcl = np.ascontiguousarray(
        input.transpose(0, 2, 3, 1).astype(np.float16)
    ).reshape(B * H * W, C)

    T, acols = plan["T"], plan["acols"]
    caps, pos, slots = plan["caps"], plan["pos"], plan["slots"]

    in_maps = []
    for k in range(N_CORES):
        logical = np.zeros(T * CH, np.int32)
        a_arr = np.zeros((128, acols), np.float16)
        for (t, r, aoff, st, sp) in plan["blocks"]:
            gidx, A = mats[slots[k][r]]
            npix = len(gidx)
            lo = max(t * CH, pos[r])
            hi = min((t + 1) * CH, pos[r] + npix)
            if hi <= lo:
                continue
            i0 = lo - pos[r]
            i1 = hi - pos[r]
            logical[lo:hi] = gidx[i0:i1]
            a_arr[lo - t * CH:hi - t * CH, aoff:aoff + NJ] = (
                A[i0:i1].astype(np.float16)
            )
        idx16 = np.tile(logical.astype(np.int16).reshape(-1, 16).T, (8, 1))
        in_maps.append({"fcl": fcl, "amat": a_arr, "pidx": idx16})

    res = run_bass_kernel_spmd(nc, in_maps, core_ids=list(range(N_CORES)))

    out_full = np.empty((N_ROIS, C, P, P), f32)
    for k in range(N_CORES):
        arr = res.results[k]["out"].astype(f32)  # (128, RPC, 2, 49)
        t = arr.transpose(1, 2, 0, 3).reshape(RPC, C, P, P)
        for r in range(RPC):
            roi = slots[k][r]
            if len(mats[roi][0]) == 0 and caps[r] > 0:
                out_full[roi] = 0.0
            elif caps[r] == 0:
                out_full[roi] = 0.0
            else:
                out_full[roi] = t[r]
    return out_full
